# revision 12
# baseline (speedup 1.0000x reference)
"""Multi-head attention (B=4, S=2048, D=1024, H=16) on 8 Trainium2 NeuronCores.

Sharding: hybrid data x tensor parallel. Core c = (batch b = c//2,
head-group g = c%2). Each core owns one batch's tokens (2048) and 8 heads
(a 512-wide slice of the model dim, 4 head-pairs "hp" of 128 dims). It
computes Q/K/V projections for its slice, causal attention for its 8 heads,
and a partial output projection (contraction over its 512 x-dims). The host
sums core pairs (b,0)+(b,1) and adds b_o. vs pure head-TP this cuts per-core
input DMA 4x (12 MB vs 48 MB).

All matmuls run in bf16 (full PE rate) with fp32 PSUM accumulation; softmax
runs without max-subtraction (scores are O(10), exp stays in range).

On-device layouts (tokens on the free axis unless noted):
  QT/KT: [128 head-pair dims, 4 hp, 2048 tokens] bf16. Head h of a pair
         occupies partitions 64h:64h+64, which row-packs the two heads'
         scores matmuls (K=64 each) into concurrent PE row-groups.
  VA:    [128 token-chunk, 16 chunks, 4 hp, 256]; per hp cols 0:64 = head0
         V dims, 64:128 = ones, 128:192 = head1 V, 192:256 = ones (the ones
         give replicated softmax row-sums for free). V is projected directly
         in transposed form (tokens on partitions, 512 vd streamed) - no PE
         transposes needed.
  Scores are computed transposed, S.T = [k-tokens, q-tokens]; both heads of
  a pair land in one [128, 1024] PSUM tile (2 banks) so a single ScalarE
  ACTIVATE exponentiates both (halves the per-call ACT overhead, which
  dominated the baseline's critical path).
  Projection / output-projection matmuls are interleaved into the attention
  stream at ~1.7us granularity so the PE stays dense while ScalarE works.
"""

import sys
import types

sys.path.insert(0, "/opt/trn_rl_repo")

import numpy as np

# Optional: make run_bass_kernel_spmd(trace=True) work on images whose antenv
# lacks axon_hooks. Harmless if unavailable; kernel() defaults to trace=False.
try:  # pragma: no cover
    import antenv
    if "antenv.axon_hooks" not in sys.modules:
        from trn_agent_boot.trn_boot import _ntff_profile_via_ctypes

        _hook = _ntff_profile_via_ctypes("/opt/axon/libaxon_pjrt.so")
        _mod = types.ModuleType("antenv.axon_hooks")
        _mod.get_axon_ntff_profile_hook = lambda: _hook
        _mod.set_axon_ntff_profile_hook = lambda h: None
        sys.modules["antenv.axon_hooks"] = _mod
        antenv.axon_hooks = _mod
except Exception:
    pass

import concourse.bass as bass
import concourse.bacc as bacc
import concourse.tile as tile
import concourse.mybir as mybir
from concourse.bass_utils import run_bass_kernel_spmd

B, S, D, H = 4, 2048, 1024, 16
DK = D // H          # 64
P = 128
NT = S // 512        # 4 token tiles of 512 per core
KO = D // P          # 8 contraction chunks for Q/K/V proj
NHP = 4              # head-pairs per core (8 heads)
NKC = S // P         # 16 key chunks of 128
NCORES = 8
F16 = mybir.dt.float16
F32 = mybir.dt.float32

TRACE = False        # set by test.py to capture an NTFF profile
LAST_RESULT = None   # BassKernelResults of the most recent run

MM_DT = mybir.dt.bfloat16

_NC = None


def _np_mm_dt():
    import ml_dtypes
    return ml_dtypes.bfloat16


def _build():
    nc = bacc.Bacc("TRN2", target_bir_lowering=False, debug=False,
                   num_devices=NCORES)

    qT_d = nc.dram_tensor("qT", [NT, P, KO, 512], MM_DT, kind="ExternalInput")
    kT_d = nc.dram_tensor("kT", [NT, P, KO, 512], MM_DT, kind="ExternalInput")
    vT_d = nc.dram_tensor("vT", [NT, P, KO, 512], MM_DT, kind="ExternalInput")
    wq_d = nc.dram_tensor("wq", [NHP, P, KO, P], MM_DT, kind="ExternalInput")
    wk_d = nc.dram_tensor("wk", [NHP, P, KO, P], MM_DT, kind="ExternalInput")
    wv_d = nc.dram_tensor("wv", [P, KO, 512], MM_DT, kind="ExternalInput")
    wo_d = nc.dram_tensor("wo", [P, NHP, KO, P], MM_DT, kind="ExternalInput")
    mk_d = nc.dram_tensor("masks", [P, 2, P], MM_DT, kind="ExternalInput")
    out_d = nc.dram_tensor("out", [KO, P, NT, 512], F16, kind="ExternalOutput")

    with tile.TileContext(nc) as tc:
        with (
            tc.tile_pool(name="const", bufs=1) as const,
            tc.tile_pool(name="persist", bufs=1) as persist,
            tc.tile_pool(name="stream", bufs=2) as stream,
            tc.tile_pool(name="epool", bufs=6) as epool,
            tc.tile_pool(name="rpool", bufs=2) as rpool,
            tc.tile_pool(name="ostage", bufs=4) as ostage,
            tc.tile_pool(name="pp", bufs=2, space="PSUM") as pp,
            tc.tile_pool(name="scp", bufs=2, space="PSUM") as scp,
            tc.tile_pool(name="opp", bufs=2, space="PSUM") as opp,
        ):
            # Weights go through the GpSimd SWDGE queue, in consumption
            # order and hp-sliced, so they stream in parallel with the
            # sync-queue input tiles and the first proj matmul can start
            # as early as possible (the runtime's ~9us DMA preamble plus
            # one 256 KiB weight slice + one input tile).
            wq_t = const.tile([P, NHP, KO, P], MM_DT, tag="wq")
            wk_t = const.tile([P, NHP, KO, P], MM_DT, tag="wk")
            wv_t = const.tile([P, KO, 512], MM_DT, tag="wv")
            wo_t = const.tile([P, NHP, KO, P], MM_DT, tag="wo")
            mk_t = const.tile([P, 2, P], MM_DT, tag="mk")
            for hp in range(NHP):
                nc.gpsimd.dma_start(wq_t[:, hp], wq_d.ap()[hp])
                nc.gpsimd.dma_start(wk_t[:, hp], wk_d.ap()[hp])
            nc.gpsimd.dma_start(wv_t[:], wv_d.ap())
            nc.gpsimd.dma_start(wo_t[:], wo_d.ap())
            nc.gpsimd.dma_start(mk_t[:], mk_d.ap())

            QT = persist.tile([P, NHP, S], MM_DT, tag="QT")
            KT = persist.tile([P, NHP, S], MM_DT, tag="KT")
            VA = persist.tile([P, NKC, NHP, 256], MM_DT, tag="VA")
            XT = persist.tile([P, NHP, S], MM_DT, tag="XT")

            # ones columns for the row-sum trick
            nc.vector.memset(VA[:, :, :, DK:P], 1.0)
            nc.vector.memset(VA[:, :, :, P + DK:2 * P], 1.0)

            ins = {}

            def load_tile_inputs(tt):
                """Issue DMA for token tile tt's q/k/v input (done early so
                the proj matmuls emitted later never stall the PE queue)."""
                for nm, dram in (("q", qT_d), ("k", kT_d), ("v", vT_d)):
                    t = stream.tile([P, KO, 512], MM_DT, tag=f"{nm}in",
                                    name=f"{nm}in")
                    nc.sync.dma_start(t[:], dram.ap()[tt])
                    ins[(nm, tt)] = t

            def qk_proj_unit(tt, which, hp):
                """One head-pair's Q or K projection for token tile tt."""
                xin = ins[(which, tt)]
                w_t = wq_t if which == "q" else wk_t
                dst = QT if which == "q" else KT
                ps = pp.tile([P, 512], F32, tag="pp", name="ps")
                for ko in range(KO):
                    nc.tensor.matmul(ps[:], w_t[:, hp, ko, :], xin[:, ko, :],
                                     start=(ko == 0), stop=(ko == KO - 1))
                nc.vector.tensor_copy(dst[:, hp, bass.ts(tt, 512)], ps[:])

            def v_proj_unit(tt, c):
                """V projection for token chunk tt*4+c, directly transposed:
                out [128 tokens, 512 vd] with the input chunk stationary."""
                vin = ins[("v", tt)]
                psv = pp.tile([P, 512], F32, tag="pp", name="psv")
                for ko in range(KO):
                    nc.tensor.matmul(psv[:], vin[:, ko, bass.ts(c, P)],
                                     wv_t[:, ko, :],
                                     start=(ko == 0), stop=(ko == KO - 1))
                kc = tt * 4 + c
                nc.vector.tensor_copy(
                    VA[:, kc].rearrange("p hp (h x) -> p hp h x", h=2)[:, :, :, 0:DK],
                    psv[:].rearrange("p (hp h x) -> p hp h x", hp=NHP, h=2))

            def oproj_unit(tt, mo, tail=False):
                """Output dims [mo*128, mo*128+128) of token tile tt's output
                projection (contraction over all 4 head-pairs). In the tail
                the PSUM evacuation runs on ScalarE (idle there) so the PE
                stream stays dense."""
                pso = pp.tile([P, 512], F32, tag="pp", name="pso")
                for hp in range(NHP):
                    nc.tensor.matmul(pso[:], wo_t[:, hp, mo, :],
                                     XT[:, hp, bass.ts(tt, 512)],
                                     start=(hp == 0), stop=(hp == NHP - 1))
                ost = ostage.tile([P, 512], F16, tag="ost")
                if tail:
                    nc.scalar.copy(ost[:], pso[:])
                else:
                    nc.vector.tensor_copy(ost[:], pso[:])
                nc.sync.dma_start(out_d.ap()[mo, :, tt, :], ost[:])

            class Pacer:
                """Drains filler closures evenly across a row's chunks."""
                def __init__(self):
                    self.fillers = []
                    self.total = 1
                    self.done = 0
                    self.drained = 0

                def set_row(self, fillers, total_chunks):
                    self.fillers = fillers
                    self.total = max(total_chunks, 1)
                    self.done = 0
                    self.drained = 0

                def tick(self):
                    self.done += 1
                    want = (self.done * len(self.fillers)) // self.total
                    while self.drained < want:
                        self.fillers[self.drained]()
                        self.drained += 1

                def flush(self):
                    while self.drained < len(self.fillers):
                        self.fillers[self.drained]()
                        self.drained += 1

            pacer = Pacer()

            def attention(hp, qt):
                """One 512-query tile of causal attention for head-pair hp.

                Both heads' scores run as concurrent PE row-groups (K=64 at
                partition bases 0 and 64) into one [128, 2 heads, 512] fp32
                PSUM tile (2 banks), so ONE ScalarE ACTIVATE exponentiates
                both heads (the per-call ACT overhead paced the baseline).
                Software-pipelined: chunk g's scores + exp are emitted
                before chunk g-1's attn@V matmuls. (2-chunk exp batching is
                impossible on TRN2: matmul PSUM output must be fp32 and the
                8 PSUM banks are fully committed.)
                """
                qcols = bass.ts(qt, 512)
                nkc = 4 * qt + 4
                ops = [opp.tile([P, 512], F32, tag="op", name=f"op{h}")
                       for h in range(2)]
                groups = [((kc, max(kc - 4 * qt, 0) * P),)
                          for kc in range(nkc)]
                e_prev = [None]

                def scores_exp(g):
                    ssc = scp.tile([P, 2, 512], F32, tag="sc", name="ssc")
                    (kc, co), = g
                    w = 512 - co
                    for h in range(2):
                        rb = h * DK
                        nc.tensor.matmul(
                            ssc[:, h, bass.ds(co, w)],
                            KT[rb:rb + DK, hp, bass.ds(kc * P, P)],
                            QT[rb:rb + DK, hp, bass.ds(qt * 512 + co, w)],
                            start=True, stop=True)
                    e_t = epool.tile([P, 2, 512], MM_DT, tag="e", name="e_t")
                    nc.scalar.activation(
                        e_t[:, :, co:], ssc[:, :, co:],
                        mybir.ActivationFunctionType.Exp, scale=0.125)
                    if kc >= 4 * qt:
                        # diagonal block: zero the q < k triangle
                        nc.vector.tensor_mul(e_t[:, :, co:co + P],
                                             e_t[:, :, co:co + P], mk_t[:])
                    e_prev[0] = (e_t, g)

                def attn_group(e_t, g):
                    (kc, co), = g
                    for h in range(2):
                        nc.tensor.matmul(
                            ops[h][:, co:],
                            VA[:, kc, hp, bass.ts(h, P)],
                            e_t[:, h, bass.ds(co, 512 - co)],
                            start=(kc == 0), stop=(kc == nkc - 1))
                    pacer.tick()

                scores_exp(groups[0])
                for gi in range(1, len(groups)):
                    e_t, g = e_prev[0]
                    scores_exp(groups[gi])
                    attn_group(e_t, g)
                attn_group(*e_prev[0])

                for h in range(2):
                    s_t = rpool.tile([DK, 512], F32, tag="s", name="s_t")
                    nc.vector.tensor_copy(s_t[:], ops[h][DK:P, :])
                    r_t = rpool.tile([DK, 512], F32, tag="r", name="r_t")
                    nc.vector.reciprocal_approx_fast(r_t[:], s_t[:])
                    nc.vector.tensor_mul(XT[h * DK:(h + 1) * DK, hp, qcols],
                                         ops[h][0:DK, :], r_t[:])

            def interleave(a, b):
                """Merge two filler lists at even fractional spacing."""
                out = [(i / max(len(a), 1), f) for i, f in enumerate(a)]
                out += [((i + 0.5) / max(len(b), 1), f)
                        for i, f in enumerate(b)]
                return [f for _, f in sorted(out, key=lambda t: t[0])]

            # Prologue: tile 0's projections run unpaced (nothing else to
            # overlap with yet); tiles 1-3 and each row's output projection
            # are drained as fillers inside the next row's attention stream.
            # Input DMAs are issued a full row ahead of their consumers so
            # the PE queue never stalls on them.
            load_tile_inputs(0)
            load_tile_inputs(1)
            for which in ("q", "k"):
                for hp in range(NHP):
                    qk_proj_unit(0, which, hp)
            for c in range(4):
                v_proj_unit(0, c)

            # oproj(t) drains two rows later (row t+2) where the growing
            # causal-attention ACT load leaves PE slack; rows 0-1 are
            # already PE-bound. Row 3 keeps 6 of oproj(t2); the last 2
            # drain after row 3 to cover its final normalize latency.
            oproj_rows = {2: [(0, mo) for mo in range(KO)],
                          3: [(1, mo) for mo in range(KO)]
                          + [(2, mo) for mo in range(6)]}
            for qt in range(NT):
                if qt + 2 < NT:
                    load_tile_inputs(qt + 2)
                proj_f, oproj_f = [], []
                if qt + 1 < NT:
                    for i in range(NHP):
                        proj_f.append(
                            lambda hp=i: qk_proj_unit(qt + 1, "q", hp))
                        proj_f.append(
                            lambda hp=i: qk_proj_unit(qt + 1, "k", hp))
                        proj_f.append(lambda c=i: v_proj_unit(qt + 1, c))
                for t, mo in oproj_rows.get(qt, []):
                    oproj_f.append(lambda t=t, m=mo: oproj_unit(t, m))
                pacer.set_row(interleave(proj_f, oproj_f), NHP * (4 * qt + 4))
                for hp in range(NHP):
                    attention(hp, qt)
                pacer.flush()

            for mo in range(6, KO):
                oproj_unit(2, mo, tail=True)
            for mo in range(KO):
                oproj_unit(NT - 1, mo, tail=True)

    nc.compile()
    return nc


def _get_nc():
    global _NC
    if _NC is None:
        _NC = _build()
    return _NC


def _to_tiled_T(x2):
    """[S, D] fp32 -> [NT, 128, KO, 512] bf16 with x[t, d] at
    [t//512, d%128, d//128, t%512]."""
    xh = x2.astype(_np_mm_dt())
    return np.ascontiguousarray(
        xh.reshape(NT, 512, KO, P).transpose(0, 3, 2, 1))


def _weight_qk(w_slice):
    """[512 out, 1024 in] -> [NHP, 128 p, KO, 128 m] with W[m_full, d] at
    [m_full//128, d%128, d//128, m_full%128]."""
    return np.ascontiguousarray(
        w_slice.T.reshape(KO, P, NHP, P).transpose(2, 1, 0, 3)
    ).astype(_np_mm_dt())


def kernel(q, k, v, mask, W_q, W_k, W_v, W_o, b_o):
    global LAST_RESULT
    nc = _get_nc()

    q = np.asarray(q, np.float32)
    k = np.asarray(k, np.float32)
    v = np.asarray(v, np.float32)
    W_q = np.asarray(W_q, np.float32)
    W_k = np.asarray(W_k, np.float32)
    W_v = np.asarray(W_v, np.float32)
    W_o = np.asarray(W_o, np.float32)

    p_idx = np.arange(P)[:, None]
    f_idx = np.arange(P)[None, :]
    masks = np.broadcast_to(
        (f_idx >= p_idx)[:, None, :], (P, 2, P)).astype(_np_mm_dt())
    masks = np.ascontiguousarray(masks)

    qT = [_to_tiled_T(q[b]) for b in range(B)]
    kT = [_to_tiled_T(k[b]) for b in range(B)]
    vT = [_to_tiled_T(v[b]) for b in range(B)]

    in_maps = []
    for c in range(NCORES):
        b, g = c // 2, c % 2
        gs = slice(g * 512, (g + 1) * 512)
        # wv: [128 p(in%128), KO, 512 out] = W_v[out, in].T tiled
        wv = np.ascontiguousarray(
            W_v[gs, :].T.reshape(KO, P, 512).transpose(1, 0, 2)
        ).astype(_np_mm_dt())
        # wo: [128 k(x%128), NHP, KO mo, 128 m] = W_o[mo*128+m, g*512+hp*128+k]
        wo = np.ascontiguousarray(
            W_o[:, gs].reshape(KO, P, NHP, P).transpose(3, 2, 0, 1)
        ).astype(_np_mm_dt())
        in_maps.append({
            "qT": qT[b], "kT": kT[b], "vT": vT[b], "masks": masks,
            "wq": _weight_qk(W_q[gs, :]),
            "wk": _weight_qk(W_k[gs, :]),
            "wv": wv, "wo": wo,
        })

    res = run_bass_kernel_spmd(nc, in_maps, core_ids=list(range(NCORES)),
                               trace=TRACE)
    LAST_RESULT = res

    out = np.empty((B, S, D), np.float32)
    b_o32 = np.asarray(b_o, np.float32)
    for b in range(B):
        p0 = res.results[2 * b]["out"].reshape(D, S).astype(np.float32)
        p1 = res.results[2 * b + 1]["out"].reshape(D, S).astype(np.float32)
        out[b] = (p0 + p1).T + b_o32
    return out
